# revision 8
# baseline (speedup 1.0000x reference)
"""CrossScan Trainium2 kernel.

Input  x: (8, 192, 128, 128) f32  [B, C, H, W]
Output:   (4, 8, 16384, 192) f32  [scan, B, H*W, C]

Sharding: pure data-parallel over B (one batch per NeuronCore, 8 cores).

Per core: the four scans are all (spatial, C) transposes of the local
(C, H, W) map:
  scan0[h*W+w, c] = x[c, h, w]
  scan1[h*W+w, c] = x[c, h, W-1-w]   (= scan0 tile with rows reversed)
  scan2[w*H+h, c] = x[c, h, w]
  scan3[w*H+h, c] = x[c, H-1-h, w]   (= scan2 tile with rows reversed)

Strategy: load x fully into SBUF (12.6 MB), then for each of 128 h-rows
PE-transpose the (C, W) slab into a (W, C) tile, and for each of 128
w-columns PE-transpose the (C, H) slab into an (H, C) tile.  Each tile
is copied PSUM->SBUF once and DMA'd to DRAM twice: once to the ascending
output block, once to the flip-variant output through a reversed
(negative-stride) DRAM view.  Every store is a 98 KB DMA with 768 B
descriptors.
"""

import numpy as np

import concourse.bacc as bacc
import concourse.bass as bass
import concourse.mybir as mybir
import concourse.tile as tile
from concourse import masks
from concourse.bass_utils import run_bass_kernel_spmd

B, C, H, W = 8, 192, 128, 128
HW = H * W
N_CORES = 8

_cached_nc = None


def _build():
    global _cached_nc
    if _cached_nc is not None:
        return _cached_nc

    f32 = mybir.dt.float32
    nc = bacc.Bacc("TRN2", target_bir_lowering=False, debug=False, num_devices=N_CORES)
    x = nc.dram_tensor("x", [C, H, W], f32, kind="ExternalInput").ap()
    out = nc.dram_tensor("out", [4, HW, C], f32, kind="ExternalOutput").ap()

    with tile.TileContext(nc) as tc:
        with (
            tc.tile_pool(name="const", bufs=1) as constp,
            tc.tile_pool(name="xin", bufs=1) as xin,
            tc.tile_pool(name="psum", bufs=8, space="PSUM") as psp,
            tc.tile_pool(name="stage", bufs=8) as stp,
            tc.tile_pool(name="gather", bufs=4) as gp,
        ):
            ident = constp.tile([128, 128], f32)
            masks.make_identity(nc, ident[:])

            # Whole input resident in SBUF, split into the two C chunks.
            T0 = xin.tile([128, HW], f32, tag="T0")
            T1 = xin.tile([64, HW], f32, tag="T1")
            xflat = x.rearrange("c h w -> c (h w)")
            # Single DMA per chunk: consumers of T0/T1 then wait on at most
            # two DMA semaphore lanes (HW limits sync-waits per instruction).
            nc.sync.dma_start(out=T0[:], in_=xflat[0:128, :])
            nc.sync.dma_start(out=T1[:], in_=xflat[128:192, :])

            T0v = T0[:].rearrange("c (h w) -> c h w", w=W)
            T1v = T1[:].rearrange("c (h w) -> c h w", w=W)

            def emit_block(in0, in1, dst, flip=False):
                if flip:
                    # Engines can't take negative-stride matmul operands;
                    # materialize the flipped slab with DVE copies first.
                    g0 = gp.tile([128, 128], f32, tag="g0")
                    g1 = gp.tile([64, 128], f32, tag="g1")
                    nc.vector.tensor_copy(g0[:], in0)
                    nc.vector.tensor_copy(g1[:], in1)
                    in0, in1 = g0[:], g1[:]
                ps = psp.tile([128, C], f32, tag="ps")
                nc.tensor.transpose(ps[:, 0:128], in0, ident[:])
                nc.tensor.transpose(ps[:, 128:192], in1, ident[:64, :64])
                st = stp.tile([128, C], f32, tag="st")
                nc.vector.tensor_copy(st[:], ps[:])
                nc.sync.dma_start(out=dst, in_=st[:])

            for h in range(H):
                # scan0[h*W + w, c] = x[c, h, w]
                emit_block(
                    T0v[:, h, :], T1v[:, h, :], out[0, h * W : (h + 1) * W, :]
                )
                # scan1[h*W + w, c] = x[c, h, W-1-w]
                emit_block(
                    T0v[:, h, ::-1],
                    T1v[:, h, ::-1],
                    out[1, h * W : (h + 1) * W, :],
                    flip=True,
                )
            for w in range(W):
                # scan2[w*H + h, c] = x[c, h, w]
                emit_block(
                    T0v[:, :, w], T1v[:, :, w], out[2, w * H : (w + 1) * H, :]
                )
                # scan3[w*H + h, c] = x[c, H-1-h, w]
                emit_block(
                    T0v[:, ::-1, w],
                    T1v[:, ::-1, w],
                    out[3, w * H : (w + 1) * H, :],
                    flip=True,
                )

    nc.compile()
    _cached_nc = nc
    return nc


def _run(x, trace=False, **kwargs):
    nc = _build()
    x = np.ascontiguousarray(np.asarray(x, dtype=np.float32))
    in_maps = [{"x": x[b]} for b in range(B)]
    res = run_bass_kernel_spmd(nc, in_maps, list(range(N_CORES)), trace=trace, **kwargs)
    full = np.stack([res.results[b]["out"] for b in range(B)], axis=1)
    return full, res


def kernel(x):
    full, _ = _run(x, trace=False)
    return full


# revision 12
# speedup vs baseline: 26.3307x; 26.3307x over previous
"""CrossScan Trainium2 kernel.

Input  x: (8, 192, 128, 128) f32  [B, C, H, W]
Output:   (4, 8, 16384, 192) f32  [scan, B, H*W, C]

Sharding: pure data-parallel over B (one batch per NeuronCore, 8 cores).

Per core: the four scans are all (spatial, C) transposes of the local
(C, H, W) map:
  scan0[h*W+w, c] = x[c, h, w]
  scan1[h*W+w, c] = x[c, h, W-1-w]   (= scan0 tile with rows reversed)
  scan2[w*H+h, c] = x[c, h, w]
  scan3[w*H+h, c] = x[c, H-1-h, w]   (= scan2 tile with rows reversed)

Strategy: load x fully into SBUF (12.6 MB), then for each of 128 h-rows
PE-transpose the (C, W) slab into a (W, C) tile, and for each of 128
w-columns PE-transpose the (C, H) slab into an (H, C) tile.  Each tile
is copied PSUM->SBUF once and DMA'd to DRAM twice: once to the ascending
output block, once to the flip-variant output through a reversed
(negative-stride) DRAM view.  Every store is a 98 KB DMA with 768 B
descriptors.
"""

import numpy as np

import concourse.bacc as bacc
import concourse.bass as bass
import concourse.mybir as mybir
import concourse.tile as tile
from concourse import masks
from concourse.bass_utils import run_bass_kernel_spmd

B, C, H, W = 8, 192, 128, 128
HW = H * W
N_CORES = 8

_cached_nc = {}


def _build(loop_iters=None):
    """Build the per-core program.  loop_iters wraps the whole body in an
    on-device For_i loop (used only for timing: amortizes host dispatch)."""
    global _cached_nc
    if loop_iters in _cached_nc:
        return _cached_nc[loop_iters]

    import contextlib

    f32 = mybir.dt.float32
    nc = bacc.Bacc("TRN2", target_bir_lowering=False, debug=False, num_devices=N_CORES)
    x = nc.dram_tensor("x", [C, H, W], f32, kind="ExternalInput").ap()
    out = nc.dram_tensor("out", [4, HW, C], f32, kind="ExternalOutput").ap()

    with tile.TileContext(nc) as tc:
        with (
            tc.tile_pool(name="const", bufs=1) as constp,
            tc.tile_pool(name="xin", bufs=1) as xin,
            tc.tile_pool(name="psum", bufs=8, space="PSUM") as psp,
            tc.tile_pool(name="stage", bufs=8) as stp,
            tc.tile_pool(name="gather", bufs=4) as gp,
        ):
            ident = constp.tile([128, 128], f32)
            masks.make_identity(nc, ident[:])

            loop_cm = (
                tc.For_i(0, loop_iters, 1) if loop_iters else contextlib.nullcontext()
            )
            with loop_cm:
                _emit_body(nc, tc, x, out, ident, xin, psp, stp, gp, f32)

    nc.compile()
    _cached_nc[loop_iters] = nc
    return nc


def _emit_body(nc, tc, x, out, ident, xin, psp, stp, gp, f32):
    # Whole input resident in SBUF, split into the two C chunks.
    T0 = xin.tile([128, HW], f32, tag="T0")
    T1 = xin.tile([64, HW], f32, tag="T1")
    xflat = x.rearrange("c h w -> c (h w)")
    # Single DMA per chunk: consumers of T0/T1 then wait on at most
    # two DMA semaphore lanes (HW limits sync-waits per instruction).
    nc.sync.dma_start(out=T0[:], in_=xflat[0:128, :])
    nc.sync.dma_start(out=T1[:], in_=xflat[128:192, :])

    T0v = T0[:].rearrange("c (h w) -> c h w", w=W)
    T1v = T1[:].rearrange("c (h w) -> c h w", w=W)

    def emit_block(in0, in1, dst, flip=False):
        if flip:
            # Engines can't take negative-stride matmul operands;
            # materialize the flipped slab with DVE copies first.
            g0 = gp.tile([128, 128], f32, tag="g0")
            g1 = gp.tile([64, 128], f32, tag="g1")
            nc.vector.tensor_copy(g0[:], in0)
            nc.vector.tensor_copy(g1[:], in1)
            in0, in1 = g0[:], g1[:]
        ps = psp.tile([128, C], f32, tag="ps")
        nc.tensor.transpose(ps[:, 0:128], in0, ident[:])
        nc.tensor.transpose(ps[:, 128:192], in1, ident[:64, :64])
        st = stp.tile([128, C], f32, tag="st")
        nc.vector.tensor_copy(st[:], ps[:])
        nc.sync.dma_start(out=dst, in_=st[:])

    for h in range(H):
        # scan0[h*W + w, c] = x[c, h, w]
        emit_block(T0v[:, h, :], T1v[:, h, :], out[0, h * W : (h + 1) * W, :])
        # scan1[h*W + w, c] = x[c, h, W-1-w]
        emit_block(
            T0v[:, h, ::-1],
            T1v[:, h, ::-1],
            out[1, h * W : (h + 1) * W, :],
            flip=True,
        )
    for w in range(W):
        # scan2[w*H + h, c] = x[c, h, w]
        emit_block(T0v[:, :, w], T1v[:, :, w], out[2, w * H : (w + 1) * H, :])
        # scan3[w*H + h, c] = x[c, H-1-h, w]
        emit_block(
            T0v[:, ::-1, w],
            T1v[:, ::-1, w],
            out[3, w * H : (w + 1) * H, :],
            flip=True,
        )


def _run(x, trace=False, **kwargs):
    nc = _build()
    x = np.ascontiguousarray(np.asarray(x, dtype=np.float32))
    in_maps = [{"x": x[b]} for b in range(B)]
    res = run_bass_kernel_spmd(nc, in_maps, list(range(N_CORES)), trace=trace, **kwargs)
    full = np.stack([res.results[b]["out"] for b in range(B)], axis=1)
    return full, res


def kernel(x):
    full, _ = _run(x, trace=False)
    return full


# revision 13
# speedup vs baseline: 37.3655x; 1.4191x over previous
"""CrossScan Trainium2 kernel.

Input  x: (8, 192, 128, 128) f32  [B, C, H, W]
Output:   (4, 8, 16384, 192) f32  [scan, B, H*W, C]

Sharding: pure data-parallel over B (one batch per NeuronCore, 8 cores).

Per core: the four scans are all (spatial, C) transposes of the local
(C, H, W) map:
  scan0[h*W+w, c] = x[c, h, w]
  scan1[h*W+w, c] = x[c, h, W-1-w]   (= scan0 tile with rows reversed)
  scan2[w*H+h, c] = x[c, h, w]
  scan3[w*H+h, c] = x[c, H-1-h, w]   (= scan2 tile with rows reversed)

Strategy: keep x resident in SBUF.  For each spatial block of 128
positions, PE-transpose the (C, 128) slab into a (128, C) tile (two
matmuls: C = 128 + 64).  The flipped variants are produced by a second
PE matmul against the anti-diagonal exchange matrix J (out = J.T @ st
reverses the partition axis) — DMA/matmul operands cannot have negative
strides, but J is just data.  Stores are batched 4 spatial blocks per
dma_start (HWDGE descriptor-generation cost is a fixed ~625 ns per DMA
instruction, so fewer+bigger DMAs win; each still uses 768 B
descriptors, which run at full DMA-bus rate).
"""

import numpy as np

import concourse.bacc as bacc
import concourse.bass as bass
import concourse.mybir as mybir
import concourse.tile as tile
from concourse import masks
from concourse.bass_utils import run_bass_kernel_spmd

B, C, H, W = 8, 192, 128, 128
HW = H * W
N_CORES = 8
G = 4  # spatial blocks per store DMA

_cached_nc = {}


def _build(loop_iters=None):
    """Build the per-core program.  loop_iters wraps the whole body in an
    on-device For_i loop (used only for timing: amortizes host dispatch)."""
    global _cached_nc
    if loop_iters in _cached_nc:
        return _cached_nc[loop_iters]

    import contextlib

    f32 = mybir.dt.float32
    nc = bacc.Bacc("TRN2", target_bir_lowering=False, debug=False, num_devices=N_CORES)
    x = nc.dram_tensor("x", [C, H, W], f32, kind="ExternalInput").ap()
    out = nc.dram_tensor("out", [4, HW, C], f32, kind="ExternalOutput").ap()

    with tile.TileContext(nc) as tc:
        with (
            tc.tile_pool(name="const", bufs=1) as constp,
            tc.tile_pool(name="xin", bufs=1) as xin,
            tc.tile_pool(name="psum", bufs=4, space="PSUM") as psp,
            tc.tile_pool(name="psumf", bufs=4, space="PSUM") as psfp,
            tc.tile_pool(name="stage", bufs=6) as stp,
        ):
            ident = constp.tile([128, 128], f32)
            masks.make_identity(nc, ident[:])
            # Exchange (anti-diagonal) matrix: J[x, y] = 1 iff x + y = 127.
            exch = constp.tile([128, 128], f32)
            nc.gpsimd.memset(exch[:], 0.0)
            nc.gpsimd.affine_select(
                out=exch[:],
                in_=exch[:],
                compare_op=mybir.AluOpType.not_equal,
                fill=1.0,
                base=-127,
                pattern=[[1, 128]],
                channel_multiplier=1,
            )

            loop_cm = (
                tc.For_i(0, loop_iters, 1) if loop_iters else contextlib.nullcontext()
            )
            with loop_cm:
                _emit_body(nc, tc, x, out, ident, exch, xin, psp, psfp, stp, f32)

    nc.compile()
    _cached_nc[loop_iters] = nc
    return nc


def _emit_body(nc, tc, x, out, ident, exch, xin, psp, psfp, stp, f32):
    # Whole input resident in SBUF, split into the two C chunks.
    T0 = xin.tile([128, HW], f32, tag="T0")
    T1 = xin.tile([64, HW], f32, tag="T1")
    xflat = x.rearrange("c h w -> c (h w)")
    # Single DMA per chunk: consumers of T0/T1 then wait on at most
    # two DMA semaphore lanes (HW limits sync-waits per instruction).
    nc.sync.dma_start(out=T0[:], in_=xflat[0:128, :])
    nc.sync.dma_start(out=T1[:], in_=xflat[128:192, :])

    T0v = T0[:].rearrange("c (h w) -> c h w", w=W)
    T1v = T1[:].rearrange("c (h w) -> c h w", w=W)

    def emit_group(slabs, dst_fwd, dst_flip):
        """slabs: G (in0, in1) C-chunk pairs; writes G fwd blocks + G flipped."""
        # Forward transposes: two blocks per PSUM tile (384 f32 = one bank).
        pss = []
        for half in range(G // 2):
            ps = psp.tile([128, 2 * C], f32, tag="ps")
            for j in range(2):
                in0, in1 = slabs[half * 2 + j]
                nc.tensor.transpose(ps[:, j * C : j * C + 128], in0, ident[:])
                nc.tensor.transpose(ps[:, j * C + 128 : (j + 1) * C], in1, ident[:64, :64])
            pss.append(ps)
        st = stp.tile([128, G * C], f32, tag="st")
        for half, ps in enumerate(pss):
            nc.vector.tensor_copy(st[:, half * 2 * C : (half + 1) * 2 * C], ps[:])
        nc.sync.dma_start(out=dst_fwd, in_=st[:])

        # Flipped blocks: reverse the partition axis with J (out = J.T @ st).
        stf = stp.tile([128, G * C], f32, tag="st")
        for half in range(G // 2):
            psf = psfp.tile([128, 2 * C], f32, tag="psf")
            nc.tensor.matmul(
                psf[:], exch[:], st[:, half * 2 * C : (half + 1) * 2 * C]
            )
            nc.vector.tensor_copy(stf[:, half * 2 * C : (half + 1) * 2 * C], psf[:])
        nc.sync.dma_start(out=dst_flip, in_=stf[:])

    for h0 in range(0, H, G):
        # scan0[h*W + w, c] = x[c, h, w]; scan1 flips w within each block.
        rows = slice(h0 * W, (h0 + G) * W)
        emit_group(
            [(T0v[:, h0 + g, :], T1v[:, h0 + g, :]) for g in range(G)],
            out[0, rows, :].rearrange("(g w) c -> w g c", w=W),
            out[1, rows, :].rearrange("(g w) c -> w g c", w=W),
        )
    for w0 in range(0, W, G):
        # scan2[w*H + h, c] = x[c, h, w]; scan3 flips h within each block.
        rows = slice(w0 * H, (w0 + G) * H)
        emit_group(
            [(T0v[:, :, w0 + g], T1v[:, :, w0 + g]) for g in range(G)],
            out[2, rows, :].rearrange("(g h) c -> h g c", h=H),
            out[3, rows, :].rearrange("(g h) c -> h g c", h=H),
        )


def _run(x, trace=False, **kwargs):
    nc = _build()
    x = np.ascontiguousarray(np.asarray(x, dtype=np.float32))
    in_maps = [{"x": x[b]} for b in range(B)]
    res = run_bass_kernel_spmd(nc, in_maps, list(range(N_CORES)), trace=trace, **kwargs)
    full = np.stack([res.results[b]["out"] for b in range(B)], axis=1)
    return full, res


def kernel(x):
    full, _ = _run(x, trace=False)
    return full


# revision 17
# speedup vs baseline: 891.9056x; 23.8698x over previous
"""CrossScan Trainium2 kernel.

Input  x: (8, 192, 128, 128) f32  [B, C, H, W]
Output:   (4, 8, 16384, 192) f32  [scan, B, H*W, C]

Sharding: pure data-parallel over B (one batch per NeuronCore, 8 cores).

Per core: the four scans are all (spatial, C) transposes of the local
(C, H, W) map:
  scan0[h*W+w, c] = x[c, h, w]
  scan1[h*W+w, c] = x[c, h, W-1-w]   (= scan0 tile with rows reversed)
  scan2[w*H+h, c] = x[c, h, w]
  scan3[w*H+h, c] = x[c, H-1-h, w]   (= scan2 tile with rows reversed)

Strategy: keep x resident in SBUF.  For each spatial block of 128
positions, PE-transpose the (C, 128) slab into a (128, C) tile (two
matmuls: C = 128 + 64).  The flipped variants are produced by a second
PE matmul against the anti-diagonal exchange matrix J (out = J.T @ st
reverses the partition axis) — DMA/matmul operands cannot have negative
strides, but J is just data.  Stores are batched 4 spatial blocks per
dma_start (HWDGE descriptor-generation cost is a fixed ~625 ns per DMA
instruction, so fewer+bigger DMAs win; each still uses 768 B
descriptors, which run at full DMA-bus rate).
"""

import numpy as np

import concourse.bacc as bacc
import concourse.bass as bass
import concourse.mybir as mybir
import concourse.tile as tile
from concourse import masks
from concourse.bass_utils import run_bass_kernel_spmd

B, C, H, W = 8, 192, 128, 128
HW = H * W
N_CORES = 8
G = 4  # spatial blocks per store DMA

_cached_nc = {}


def _build(loop_iters=None, variant="", g=G):
    """Build the per-core program.  loop_iters wraps the whole body in an
    on-device For_i loop (used only for timing: amortizes host dispatch).
    variant: ''        - real kernel
             'noflip'  - skip flip matmuls/copies, store fwd tile twice
                         (timing ablation only: same bytes, half compute)
             'dual'    - alternate store DMAs between sync and scalar DGE
    """
    global _cached_nc
    key = (loop_iters, variant, g)
    if key in _cached_nc:
        return _cached_nc[key]

    import contextlib

    f32 = mybir.dt.float32
    nc = bacc.Bacc("TRN2", target_bir_lowering=False, debug=False, num_devices=N_CORES)
    x = nc.dram_tensor("x", [C, H, W], f32, kind="ExternalInput").ap()
    out = nc.dram_tensor("out", [4, HW, C], f32, kind="ExternalOutput").ap()

    with tile.TileContext(nc) as tc:
        with (
            tc.tile_pool(name="const", bufs=1) as constp,
            tc.tile_pool(name="xin", bufs=1) as xin,
            tc.tile_pool(name="psum", bufs=4, space="PSUM") as psp,
            tc.tile_pool(name="psumf", bufs=4, space="PSUM") as psfp,
            tc.tile_pool(name="stage", bufs=6) as stp,
        ):
            ident = constp.tile([128, 128], f32)
            masks.make_identity(nc, ident[:])
            # Exchange (anti-diagonal) matrix: J[x, y] = 1 iff x + y = 127.
            exch = constp.tile([128, 128], f32)
            nc.gpsimd.memset(exch[:], 0.0)
            nc.gpsimd.affine_select(
                out=exch[:],
                in_=exch[:],
                compare_op=mybir.AluOpType.not_equal,
                fill=1.0,
                base=-127,
                pattern=[[1, 128]],
                channel_multiplier=1,
            )

            loop_cm = (
                tc.For_i(0, loop_iters, 1) if loop_iters else contextlib.nullcontext()
            )
            with loop_cm:
                _emit_body(
                    nc, tc, x, out, ident, exch, xin, psp, psfp, stp, f32, variant, g
                )

    nc.compile()
    _cached_nc[key] = nc
    return nc


def _emit_body(nc, tc, x, out, ident, exch, xin, psp, psfp, stp, f32, variant="", G=G):
    # Whole input resident in SBUF, split into the two C chunks.
    T0 = xin.tile([128, HW], f32, tag="T0")
    T1 = xin.tile([64, HW], f32, tag="T1")
    xflat = x.rearrange("c h w -> c (h w)")
    # Single DMA per chunk: consumers of T0/T1 then wait on at most
    # two DMA semaphore lanes (HW limits sync-waits per instruction).
    nc.sync.dma_start(out=T0[:], in_=xflat[0:128, :])
    nc.sync.dma_start(out=T1[:], in_=xflat[128:192, :])

    if variant == "loadonly":
        # Timing ablation: loads plus one tiny store to keep output alive.
        st = stp.tile([128, G * C], f32, tag="st")
        nc.vector.tensor_copy(st[:], T0[:, : G * C])
        nc.sync.dma_start(
            out=out[0, 0 : G * W, :].rearrange("(g w) c -> w g c", w=W), in_=st[:]
        )
        return
    if variant == "storeonly":
        # Timing ablation: all 256 group stores from one constant tile.
        st = stp.tile([128, G * C], f32, tag="st")
        nc.vector.tensor_copy(st[:], T0[:, : G * C])
        for s in range(4):
            for r0 in range(0, HW, G * W):
                nc.sync.dma_start(
                    out=out[s, r0 : r0 + G * W, :].rearrange(
                        "(g w) c -> w g c", w=W
                    ),
                    in_=st[:],
                )
        return

    T0v = T0[:].rearrange("c (h w) -> c h w", w=W)
    T1v = T1[:].rearrange("c (h w) -> c h w", w=W)

    group_idx = [0]

    def emit_group(slabs, dst_fwd, dst_flip):
        """slabs: G (in0, in1) C-chunk pairs; writes G fwd blocks + G flipped."""
        group_idx[0] += 1
        store_eng = (
            nc.scalar if variant == "dual" and group_idx[0] % 2 else nc.sync
        )
        # Forward transposes: two blocks per PSUM tile (384 f32 = one bank).
        pss = []
        for half in range(G // 2):
            ps = psp.tile([128, 2 * C], f32, tag="ps")
            for j in range(2):
                in0, in1 = slabs[half * 2 + j]
                nc.tensor.transpose(ps[:, j * C : j * C + 128], in0, ident[:])
                nc.tensor.transpose(ps[:, j * C + 128 : (j + 1) * C], in1, ident[:64, :64])
            pss.append(ps)
        st = stp.tile([128, G * C], f32, tag="st")
        for half, ps in enumerate(pss):
            nc.vector.tensor_copy(st[:, half * 2 * C : (half + 1) * 2 * C], ps[:])
        store_eng.dma_start(out=dst_fwd, in_=st[:])

        if variant == "noflip":
            store_eng.dma_start(out=dst_flip, in_=st[:])
            return
        # Flipped blocks: reverse the partition axis with J (out = J.T @ st).
        stf = stp.tile([128, G * C], f32, tag="st")
        for half in range(G // 2):
            psf = psfp.tile([128, 2 * C], f32, tag="psf")
            nc.tensor.matmul(
                psf[:], exch[:], st[:, half * 2 * C : (half + 1) * 2 * C]
            )
            nc.vector.tensor_copy(stf[:, half * 2 * C : (half + 1) * 2 * C], psf[:])
        store_eng.dma_start(out=dst_flip, in_=stf[:])

    for h0 in range(0, H, G):
        # scan0[h*W + w, c] = x[c, h, w]; scan1 flips w within each block.
        rows = slice(h0 * W, (h0 + G) * W)
        emit_group(
            [(T0v[:, h0 + g, :], T1v[:, h0 + g, :]) for g in range(G)],
            out[0, rows, :].rearrange("(g w) c -> w g c", w=W),
            out[1, rows, :].rearrange("(g w) c -> w g c", w=W),
        )
    for w0 in range(0, W, G):
        # scan2[w*H + h, c] = x[c, h, w]; scan3 flips h within each block.
        rows = slice(w0 * H, (w0 + G) * H)
        emit_group(
            [(T0v[:, :, w0 + g], T1v[:, :, w0 + g]) for g in range(G)],
            out[2, rows, :].rearrange("(g h) c -> h g c", h=H),
            out[3, rows, :].rearrange("(g h) c -> h g c", h=H),
        )


def _run(x, trace=False, **kwargs):
    nc = _build()
    x = np.ascontiguousarray(np.asarray(x, dtype=np.float32))
    in_maps = [{"x": x[b]} for b in range(B)]
    res = run_bass_kernel_spmd(nc, in_maps, list(range(N_CORES)), trace=trace, **kwargs)
    full = np.stack([res.results[b]["out"] for b in range(B)], axis=1)
    return full, res


def kernel(x):
    full, _ = _run(x, trace=False)
    return full
